# revision 39
# baseline (speedup 1.0000x reference)
"""GIN encoder (5-layer GNN + BN + global pooling) on 8 TRN2 NeuronCores.

kernel(**inputs) takes FULL inputs, returns FULL [8192, 128] output.

v2 design (SPMD, one bass program, per-core data):
- Nodes in 8 contiguous 25000-row shards; activations exchanged per layer
  via CHUNKED AllGather (4 slices, overlapped with tile compute) into a
  chunk-interleaved bf16 table [200000,128]; layer 4 skips the exchange.
- Aggregation per 4-tile group into a [128,512] PSUM: gathered src rows
  via batched multi-chunk indirect DMA (12 chunks per instruction, 3D out
  AP, round-robin over 4 SWDGE queues); 0/1 selection matrices generated
  ON-CHIP per group with one DVE is_equal (broadcast AP) from dst offsets;
  self term via identity matmul from the previous-layer zbuf (batched
  stage loads).
- Delayed BN: w1 pre-scaled by s; the deg'*(W1^T t) rank-1 term lands in
  the MLP1 PSUM as one K=1 matmul per group; BN stats from Gram diagonal
  + pooled ones column, AllReduced.
- Pooling accumulated on-chip across layers (acc += lcw*s (.) pall_L),
  written once; host adds the cnt*(sum lcw*t + lcb) term and overlap-adds
  per-tile windows.
"""
import os
import numpy as np
import ml_dtypes

from concourse import bass, bacc, tile, mybir
from concourse import bass_utils

N_NODES = 200000
N_EDGES = 400000
N_FEAT = 78
DIM = 128
N_LAYERS = 5
N_GRAPHS = 8192
BN_EPS = 1e-5
NC = 8
NLOC = N_NODES // NC
NT = (NLOC + 127) // 128          # 196 tiles of 128 nodes
NG4 = (NT + 3) // 4               # 49 groups of 4 tiles
AGCH = 4                          # AllGather chunks per layer
CHROW = NLOC // AGCH              # 6250 rows per AG chunk per core
OOB = 1 << 30
PAD_DOFF = 200.0

f32 = mybir.dt.float32
bf16 = mybir.dt.bfloat16
i32 = mybir.dt.int32
Relu = mybir.ActivationFunctionType.Relu
Copy = mybir.ActivationFunctionType.Copy
Sqrt = mybir.ActivationFunctionType.Sqrt
ADD = mybir.AluOpType.add
MULT = mybir.AluOpType.mult
SUB = mybir.AluOpType.subtract
ISEQ = mybir.AluOpType.is_equal

_CACHE = {}
_LAST_RES = None


def _remap(g):
    """global node id -> row in the chunk-interleaved AG table."""
    c2 = g // NLOC
    r = g % NLOC
    k = r // CHROW
    return k * (NC * CHROW) + c2 * CHROW + (r % CHROW)


def _prep(src, dst, batch):
    order = np.argsort(dst, kind="stable")
    src_s = src[order].astype(np.int64)
    dst_s = dst[order].astype(np.int64)

    cores_e = []
    T = 0
    for c in range(NC):
        lo = c * NLOC
        m = (dst_s >= lo) & (dst_s < lo + NLOC)
        es, ed = src_s[m], dst_s[m] - lo
        cnt = np.bincount(ed // 128, minlength=NT)
        T = max(T, int(np.ceil(cnt.max() / 128)))
        cores_e.append((es, ed, cnt))

    PW = 0
    glo_all = []
    for c in range(NC):
        b = batch[c * NLOC:(c + 1) * NLOC]
        glo = np.zeros(NT, np.int64)
        for t in range(NT):
            seg = b[t * 128: min((t + 1) * 128, NLOC)]
            glo[t] = seg[0]
            PW = max(PW, int(seg[-1] - seg[0] + 1))
        glo_all.append(glo)
    PW += 1  # ones column

    percore = []
    for c in range(NC):
        es, ed, cnt = cores_e[c]
        NCH = NT * T
        idx = np.full((NCH, 128), OOB, np.int64)
        doff = np.full((NCH, 128), PAD_DOFF, np.float32)
        off = np.concatenate([[0], np.cumsum(cnt)])
        for t in range(NT):
            e0, e1 = int(off[t]), int(off[t + 1])
            r = np.arange(e1 - e0)
            idx[t * T + r // 128, r % 128] = _remap(es[e0:e1])
            doff[t * T + r // 128, r % 128] = ed[e0:e1] - t * 128
        idx = np.minimum(idx, OOB).astype(np.int32)

        degp = np.zeros(NG4 * 512, np.float32)
        dcnt = np.bincount(ed, minlength=NLOC).astype(np.float32)
        degp[:NLOC] = dcnt + 1.0
        degp = degp.reshape(NG4, 512)

        b = batch[c * NLOC:(c + 1) * NLOC]
        Sp = np.zeros((128, NT * PW), np.float32)
        glo = glo_all[c]
        for t in range(NT):
            n0, n1 = t * 128, min(t * 128 + 128, NLOC)
            p = np.arange(n1 - n0)
            Sp[p, t * PW + (b[n0:n1] - glo[t])] = 1.0
            Sp[p, t * PW + PW - 1] = 1.0

        percore.append(dict(
            idx=idx.T.copy(),                      # [128, NCH]
            doff=doff.T.astype(ml_dtypes.bfloat16).copy(),   # [128, NCH]
            degp=degp.astype(np.float32),          # [NG4, 512]
            Sp=Sp.astype(ml_dtypes.bfloat16),
            glo=glo))

    I_full = np.eye(128, dtype=np.float32)
    I_last = np.zeros((128, 128), np.float32)
    base = (NT - 1) * 128 - (NLOC - 128)   # slot offset of first last-tile row
    for j in range(NLOC - (NT - 1) * 128):
        I_last[base + j, j] = 1.0

    mask = np.zeros((128, NT), np.float32)
    for t in range(NT):
        mask[:min(128, NLOC - t * 128), t] = 1.0

    iota = np.tile(np.arange(128, dtype=np.float32), 4 * T)
    iota = np.repeat(iota.reshape(1, -1), 128, 0)

    return percore, T, PW, I_full, I_last, mask, iota


def _gather_q(nc, out_ap, in_ap, offset_ap, bounds, qnum):
    """Batched indirect row-gather with SWDGE queue selection.

    out_ap must be 3D [128, K, elem] so each 128-elem run gets its own
    index from offset_ap [128, K]."""
    g = nc.gpsimd
    out_l = g.lower_ap_dma(out_ap, for_indirect_dma=True)
    in_l = g.lower_ap_dma(in_ap, for_indirect_dma=True)
    assert len(in_l) == 1 and len(out_l) == 1
    off_l = g.lower_ap_dma(offset_ap)
    assert len(off_l) == 1
    in_l.append(off_l[0])
    ap_shape = in_ap.shape
    coef = 1
    for i in range(1, len(ap_shape)):
        coef *= ap_shape[i]
    in_l[0].dynamic_ap_info = mybir.DynamicAccessPatternInfo(
        c=0,
        actual_ap=out_ap.ap,
        indirect_dim_max_index=ap_shape[0],
        offset_expr=[
            mybir.DynamicAccessPatternOffsetExpr(
                coef=coef,
                aff_expr=mybir.DynamicAccessPatternOffsetExprAffExpr(
                    kind="IndirectArgId", arg_id=1,
                ),
            )
        ],
    )
    bc = [g.lower_val_access(g.to_reg(bounds))] if bounds is not None else []
    return g.add_instruction(
        mybir.InstDMACopy(
            name=g.bass.get_next_instruction_name(),
            queue=f"qPoolDynamic{qnum or ''}",
            mode="Copy",
            ins=in_l + bc,
            outs=out_l,
            oob_is_err=False,
            cce_op=mybir.AluOpType.bypass,
        )
    )


def _build(T, PW):
    nc = bacc.Bacc("TRN2", target_bir_lowering=False, debug=False,
                   num_devices=NC, num_swdge_queues=4)
    NCH = NT * T
    L5 = N_LAYERS
    GW = 4 * T          # gather chunks per 4-tile group

    xT = nc.dram_tensor("xT", [N_FEAT, NLOC], f32, kind="ExternalInput")
    idx_in = nc.dram_tensor("idx", [128, NCH], i32, kind="ExternalInput")
    doff_in = nc.dram_tensor("doff", [128, NCH], bf16, kind="ExternalInput")
    iota_in = nc.dram_tensor("iota", [128, GW * 128], bf16,
                             kind="ExternalInput")
    degp_in = nc.dram_tensor("degp", [NG4, 512], f32, kind="ExternalInput")
    Sp_in = nc.dram_tensor("Sp", [128, NT * PW], bf16, kind="ExternalInput")
    If_in = nc.dram_tensor("If", [128, 128], bf16, kind="ExternalInput")
    Il_in = nc.dram_tensor("Il", [128, 128], bf16, kind="ExternalInput")
    mask_in = nc.dram_tensor("mask", [128, NT], f32, kind="ExternalInput")
    iw1_in = nc.dram_tensor("iw1", [N_FEAT, DIM], f32, kind="ExternalInput")
    ib1_in = nc.dram_tensor("ib1", [DIM, 1], f32, kind="ExternalInput")
    iw2_in = nc.dram_tensor("iw2", [DIM, DIM], f32, kind="ExternalInput")
    ib2_in = nc.dram_tensor("ib2", [1, DIM], f32, kind="ExternalInput")
    w1_in = nc.dram_tensor("w1", [DIM, L5 * DIM], f32, kind="ExternalInput")
    w2_in = nc.dram_tensor("w2", [DIM, L5 * DIM], f32, kind="ExternalInput")
    b1T_in = nc.dram_tensor("b1T", [DIM, L5], f32, kind="ExternalInput")
    b2r_in = nc.dram_tensor("b2r", [1, L5 * DIM], f32, kind="ExternalInput")
    gamT_in = nc.dram_tensor("gamT", [DIM, L5], f32, kind="ExternalInput")
    betT_in = nc.dram_tensor("betT", [DIM, L5], f32, kind="ExternalInput")
    lcw_in = nc.dram_tensor("lcwb", [DIM, L5], f32, kind="ExternalInput")

    acc_out = nc.dram_tensor("acc", [DIM, NT * PW], f32,
                             kind="ExternalOutput")
    st_out = nc.dram_tensor("st", [L5 * 2, DIM], f32, kind="ExternalOutput")

    tables = [nc.dram_tensor(f"table{i}", [N_NODES, DIM], bf16,
                             kind="Internal", addr_space="Shared")
              for i in range(2)]
    zbuf = [nc.dram_tensor(f"zbuf{i}", [NLOC, DIM], bf16, kind="Internal")
            for i in range(2)]
    ar_in = nc.dram_tensor("ar_in", [DIM, 2], f32, kind="Internal")
    ar_out = nc.dram_tensor("ar_out", [DIM, 2], f32, kind="Internal",
                            addr_space="Shared")
    c_dram = nc.dram_tensor("c_dram", [DIM], f32, kind="Internal")
    RG = [list(range(NC))]

    with tile.TileContext(nc) as tc:
        with tc.tile_pool(name="const", bufs=1) as cp, \
             tc.tile_pool(name="gpool", bufs=8) as gpool, \
             tc.tile_pool(name="spool", bufs=3) as spool, \
             tc.tile_pool(name="gsp", bufs=2) as gsp, \
             tc.tile_pool(name="z1p", bufs=2) as z1p, \
             tc.tile_pool(name="y1p", bufs=2) as y1p, \
             tc.tile_pool(name="zsp", bufs=2) as zsp, \
             tc.tile_pool(name="xbp", bufs=2) as xbp, \
             tc.tile_pool(name="dgp", bufs=2) as dgp, \
             tc.tile_pool(name="pallp", bufs=2) as pallp, \
             tc.tile_pool(name="psA", bufs=2, space="PSUM") as psA, \
             tc.tile_pool(name="psB", bufs=2, space="PSUM") as psB, \
             tc.tile_pool(name="psC", bufs=2, space="PSUM") as psC, \
             tc.tile_pool(name="psG", bufs=1, space="PSUM") as psG, \
             tc.tile_pool(name="psP", bufs=1, space="PSUM") as psP:

            def ld(shape, dt_, src_ap, name):
                t_ = cp.tile(shape, dt_, name=name)
                nc.sync.dma_start(t_[:], src_ap)
                return t_

            idx_t = ld([128, NCH], i32, idx_in[:], "idx_t")
            doff_t = ld([128, NCH], bf16, doff_in[:], "doff_t")
            iota_t = ld([128, GW * 128], bf16, iota_in[:], "iota_t")
            Sp_t = ld([128, NT * PW], bf16, Sp_in[:], "Sp_t")
            If_t = ld([128, 128], bf16, If_in[:], "If_t")
            Il_t = ld([128, 128], bf16, Il_in[:], "Il_t")
            mask_t = ld([128, NT], f32, mask_in[:], "mask_t")
            iw1_t = ld([N_FEAT, DIM], f32, iw1_in[:], "iw1_t")
            ib1_t = ld([DIM, 1], f32, ib1_in[:], "ib1_t")
            iw2_t = ld([DIM, DIM], f32, iw2_in[:], "iw2_t")
            ib2_t = ld([1, DIM], f32, ib2_in[:], "ib2_t")
            iw2b = cp.tile([DIM, DIM], bf16, name="iw2b")
            nc.vector.tensor_copy(iw2b[:], iw2_t[:])
            ib2b = cp.tile([1, DIM], bf16, name="ib2b")
            nc.vector.tensor_copy(ib2b[:], ib2_t[:])
            w1_t = ld([DIM, L5 * DIM], f32, w1_in[:], "w1_t")
            w2_t = ld([DIM, L5 * DIM], f32, w2_in[:], "w2_t")
            b1T_t = ld([DIM, L5], f32, b1T_in[:], "b1T_t")
            b2r_t = ld([1, L5 * DIM], f32, b2r_in[:], "b2r_t")
            gamT_t = ld([DIM, L5], f32, gamT_in[:], "gamT_t")
            betT_t = ld([DIM, L5], f32, betT_in[:], "betT_t")
            lcw_t = ld([DIM, L5], f32, lcw_in[:], "lcw_t")

            ident_f = cp.tile([128, 128], f32, name="ident_f")
            nc.vector.tensor_copy(ident_f[:], If_t[:])
            ones_r = cp.tile([1, 128], f32, name="ones_r")
            nc.vector.memset(ones_r[:], 1.0)

            w1f = cp.tile([DIM, DIM], f32, name="w1f")
            nc.vector.tensor_copy(w1f[:], w1_t[:, 0:DIM])
            c_row = cp.tile([1, DIM], f32, name="c_row")
            acc_t = cp.tile([DIM, NT * PW], f32, name="acc_t")
            nc.vector.memset(acc_t[:], 0.0)

            for _ in range(8):
                g0 = gpool.tile([128, 128], bf16, name="g", tag="g")
                nc.vector.memset(g0[:], 0.0)

            def flush_z(zdst, ck0, ntile, zstage):
                """DMA zstage [128, ntile*128] -> zdst rows."""
                r0 = ck0 * 128
                full = min(ntile, (NLOC - r0) // 128)
                if full > 0:
                    nc.sync.dma_start(
                        zdst.ap()[r0:r0 + full * 128, :].rearrange(
                            "(j p) f -> p j f", p=128),
                        zstage[:, :full * 128].rearrange(
                            "p (j f) -> p j f", j=full))
                rem = NLOC - (r0 + full * 128)
                if 0 < rem < 128 and full < ntile:
                    nc.sync.dma_start(
                        zdst.ap()[r0 + full * 128:NLOC, :],
                        zstage[:rem, full * 128:(full + 1) * 128])

            # ---------------- ini embed -> zbuf[0], table ----------------
            ag_done = 0

            def maybe_ag(zdst, tdst, rows_done, last_layer):
                nonlocal ag_done
                if last_layer:
                    return
                while ag_done < AGCH and rows_done >= (ag_done + 1) * CHROW:
                    k = ag_done
                    nc.gpsimd.collective_compute(
                        "AllGather", mybir.AluOpType.bypass,
                        replica_groups=RG,
                        ins=[zdst.ap()[k * CHROW:(k + 1) * CHROW, :]],
                        outs=[tdst.ap()[k * NC * CHROW:
                                        (k + 1) * NC * CHROW, :]])
                    ag_done += 1

            for gidx in range(NG4):
                n0 = gidx * 512
                w = min(512, NLOC - n0)
                nt4 = (w + 127) // 128
                xb = xbp.tile([N_FEAT, 512], f32, name="xb", tag="xb")
                nc.sync.dma_start(xb[:, :w], xT[:, n0:n0 + w])
                yp = psB.tile([DIM, 512], f32, name="yp", tag="yp")
                nc.tensor.matmul(yp[:, :w], lhsT=iw1_t[:], rhs=xb[:, :w],
                                 start=True, stop=True)
                y1b = y1p.tile([DIM, 512], bf16, name="y1b", tag="y1")
                nc.scalar.activation(y1b[:, :w], yp[:, :w], Relu,
                                     bias=ib1_t[:], scale=1.0)
                zstage = zsp.tile([128, 512], bf16, name="zstage", tag="zs")
                for k in range(nt4):
                    cw = min(128, w - k * 128)
                    zp = psC.tile([128, DIM], f32, name="zp", tag="zp")
                    nc.tensor.matmul(zp[:cw, :],
                                     lhsT=y1b[:, k * 128:k * 128 + cw],
                                     rhs=iw2b[:], start=True, stop=False)
                    nc.tensor.matmul(zp[:cw, :], lhsT=ones_r[:, :cw],
                                     rhs=ib2_t[:], start=False, stop=True)
                    nc.scalar.activation(
                        zstage[:, k * 128:(k + 1) * 128], zp[:], Copy,
                        scale=mask_t[:, gidx * 4 + k:gidx * 4 + k + 1])
                flush_z(zbuf[0], gidx * 4, nt4, zstage)
                maybe_ag(zbuf[0], tables[0], min(n0 + 512, NLOC), False)

            # ---------------- layers ----------------
            for L in range(L5):
                zprev = zbuf[L % 2]
                zcur = zbuf[(L + 1) % 2]
                last_layer = (L == L5 - 1)
                ag_done = 0
                gram = psG.tile([128, 128], f32, name="gram", tag="gram")
                pall = pallp.tile([DIM, NT * PW], f32, name="pall",
                                  tag="pall")

                for gidx in range(NG4):
                    ck0 = gidx * 4
                    nt4 = min(4, NT - ck0)
                    # gathers: one indirect DMA per 128-row chunk (deep
                    # pipelining via the rotating per-chunk pool)
                    gtiles = []
                    for k in range(GW):
                        ch = ck0 * T + k
                        gt = gpool.tile([128, 128], bf16, name="g", tag="g")
                        _gather_q(
                            nc, gt[:],
                            tables[L % 2][:],
                            idx_t[:, ch:ch + 1],
                            N_NODES - 1, ch % 4)
                        gtiles.append(gt)
                    # S for this group (on-chip is_equal)
                    St = spool.tile([128, GW * 128], bf16, name="St",
                                    tag="S")
                    nc.vector.tensor_tensor(
                        out=St[:].rearrange("p (k f) -> p k f", k=GW),
                        in0=iota_t[:].rearrange("p (k f) -> p k f", k=GW),
                        in1=doff_t[:, ck0 * T: ck0 * T + GW].unsqueeze(
                            2).broadcast_to([128, GW, 128]),
                        op=ISEQ)
                    # self rows
                    gs = gsp.tile([128, 512], bf16, name="gs", tag="gs")
                    if ck0 * 128 + 512 <= NLOC:
                        nc.sync.dma_start(
                            gs[:].rearrange("p (j f) -> p j f", j=4),
                            zprev.ap()[ck0 * 128:ck0 * 128 + 512, :]
                            .rearrange("(j p) f -> p j f", p=128))
                    else:
                        for j in range(nt4):
                            t_ = ck0 + j
                            r0 = min(t_ * 128, NLOC - 128)
                            nc.sync.dma_start(
                                gs[:, j * 128:(j + 1) * 128],
                                zprev.ap()[r0:r0 + 128, :])
                    u = psA.tile([128, 512], f32, name="u", tag="u")
                    for j in range(nt4):
                        t_ = ck0 + j
                        for k in range(T):
                            nc.tensor.matmul(
                                u[:, j * 128:(j + 1) * 128],
                                lhsT=gtiles[j * T + k][:],
                                rhs=St[:, (j * T + k) * 128:
                                       (j * T + k + 1) * 128],
                                start=(k == 0), stop=False)
                        It = Il_t if t_ == NT - 1 else If_t
                        nc.tensor.matmul(
                            u[:, j * 128:(j + 1) * 128],
                            lhsT=gs[:, j * 128:(j + 1) * 128],
                            rhs=It[:], start=False, stop=True)
                    bw = nt4 * 128
                    z1b = z1p.tile([DIM, 512], f32, name="z1b", tag="z1")
                    nc.scalar.activation(z1b[:, :bw], u[:, :bw], Copy,
                                         scale=1.0)
                    yp = psB.tile([DIM, 512], f32, name="yp", tag="yp")
                    nc.tensor.matmul(yp[:, :bw], lhsT=w1f[:],
                                     rhs=z1b[:, :bw], start=True,
                                     stop=(L == 0))
                    if L > 0:
                        dg = dgp.tile([1, 512], f32, name="dg", tag="dg")
                        nc.sync.dma_start(dg[:], degp_in[gidx:gidx + 1, :])
                        nc.tensor.matmul(yp[:, :bw], lhsT=c_row[:],
                                         rhs=dg[:, :bw], start=False,
                                         stop=True)
                    y1b = y1p.tile([DIM, 512], f32, name="y1b", tag="y1")
                    nc.scalar.activation(y1b[:, :bw], yp[:, :bw], Relu,
                                         bias=b1T_t[:, L:L + 1], scale=1.0)
                    zstage = zsp.tile([128, 512], bf16, name="zstage",
                                      tag="zs")
                    for j in range(nt4):
                        ck = ck0 + j
                        zp = psC.tile([128, DIM], f32, name="zp", tag="zp")
                        nc.tensor.matmul(
                            zp[:], lhsT=y1b[:, j * 128:(j + 1) * 128],
                            rhs=w2_t[:, L * DIM:(L + 1) * DIM],
                            start=True, stop=False)
                        nc.tensor.matmul(
                            zp[:], lhsT=ones_r[:],
                            rhs=b2r_t[:, L * DIM:(L + 1) * DIM],
                            start=False, stop=True)
                        nc.scalar.activation(
                            zstage[:, j * 128:(j + 1) * 128], zp[:], Relu,
                            scale=mask_t[:, ck:ck + 1])
                        zts = zstage[:, j * 128:(j + 1) * 128]
                        pout = psP.tile([128, PW], f32, name="pout",
                                        tag="pout")
                        nc.tensor.matmul(
                            pout[:], lhsT=zts,
                            rhs=Sp_t[:, ck * PW:(ck + 1) * PW],
                            start=True, stop=True)
                        nc.vector.tensor_copy(
                            pall[:, ck * PW:(ck + 1) * PW], pout[:])
                        nc.tensor.matmul(gram[:], lhsT=zts, rhs=zts,
                                         start=(ck == 0),
                                         stop=(ck == NT - 1))
                    if not last_layer:
                        flush_z(zcur, ck0, nt4, zstage)
                        maybe_ag(zcur, tables[(L + 1) % 2],
                                 min((gidx + 1) * 512, NLOC), last_layer)

                # ---- stats
                zsum = cp.tile([DIM, 1], f32, name=f"zsum{L}")
                nc.vector.tensor_reduce(
                    out=zsum[:],
                    in_=pall[:].rearrange("p (t w) -> p t w",
                                          w=PW)[:, :, PW - 1],
                    op=ADD, axis=mybir.AxisListType.X)
                gmul = cp.tile([128, 128], f32, name=f"gmul{L}")
                nc.vector.tensor_tensor(out=gmul[:], in0=gram[:],
                                        in1=ident_f[:], op=MULT)
                zsq = cp.tile([DIM, 1], f32, name=f"zsq{L}")
                nc.vector.tensor_reduce(out=zsq[:], in_=gmul[:], op=ADD,
                                        axis=mybir.AxisListType.X)
                stats_sb = cp.tile([DIM, 2], f32, name=f"stats{L}")
                nc.vector.tensor_copy(stats_sb[:, 0:1], zsum[:])
                nc.vector.tensor_copy(stats_sb[:, 1:2], zsq[:])
                nc.sync.dma_start(ar_in.ap()[:, :], stats_sb[:])
                nc.gpsimd.collective_compute(
                    "AllReduce", ADD, replica_groups=RG,
                    ins=[ar_in.ap()], outs=[ar_out.ap()])
                arst = cp.tile([DIM, 2], f32, name=f"arst{L}")
                nc.sync.dma_start(arst[:], ar_out.ap()[:, :])
                mean = cp.tile([DIM, 1], f32, name=f"mean{L}")
                nc.vector.tensor_scalar(out=mean[:], in0=arst[:, 0:1],
                                        scalar1=1.0 / N_NODES, scalar2=None,
                                        op0=MULT)
                ex2 = cp.tile([DIM, 1], f32, name=f"ex2{L}")
                nc.vector.tensor_scalar(out=ex2[:], in0=arst[:, 1:2],
                                        scalar1=1.0 / N_NODES, scalar2=None,
                                        op0=MULT)
                m2 = cp.tile([DIM, 1], f32, name=f"m2{L}")
                nc.vector.tensor_tensor(out=m2[:], in0=mean[:], in1=mean[:],
                                        op=MULT)
                var = cp.tile([DIM, 1], f32, name=f"var{L}")
                nc.vector.tensor_tensor(out=var[:], in0=ex2[:], in1=m2[:],
                                        op=SUB)
                vare = cp.tile([DIM, 1], f32, name=f"vare{L}")
                nc.vector.tensor_scalar(out=vare[:], in0=var[:],
                                        scalar1=BN_EPS, scalar2=None,
                                        op0=ADD)
                sstd = cp.tile([DIM, 1], f32, name=f"sstd{L}")
                nc.scalar.activation(sstd[:], vare[:], Sqrt, bias=0.0,
                                     scale=1.0)
                rinv = cp.tile([DIM, 1], f32, name=f"rinv{L}")
                nc.vector.reciprocal(rinv[:], sstd[:])
                s_t = cp.tile([DIM, 1], f32, name=f"s{L}")
                nc.vector.tensor_tensor(out=s_t[:], in0=rinv[:],
                                        in1=gamT_t[:, L:L + 1], op=MULT)
                ms = cp.tile([DIM, 1], f32, name=f"ms{L}")
                nc.vector.tensor_tensor(out=ms[:], in0=mean[:], in1=s_t[:],
                                        op=MULT)
                t_t = cp.tile([DIM, 1], f32, name=f"t{L}")
                nc.vector.tensor_tensor(out=t_t[:], in0=betT_t[:, L:L + 1],
                                        in1=ms[:], op=SUB)
                nc.sync.dma_start(st_out.ap()[2 * L, :], s_t[:, 0])
                nc.sync.dma_start(st_out.ap()[2 * L + 1, :], t_t[:, 0])

                # fold pall into acc with lcw[L]*s
                fcol = cp.tile([DIM, 1], f32, name=f"fcol{L}")
                nc.vector.tensor_tensor(out=fcol[:], in0=s_t[:],
                                        in1=lcw_t[:, L:L + 1], op=MULT)
                ptmp = pallp.tile([DIM, NT * PW], f32, name="ptmp",
                                  tag="pall")
                nc.vector.tensor_scalar(out=ptmp[:], in0=pall[:],
                                        scalar1=fcol[:], scalar2=None,
                                        op0=MULT)
                nc.vector.tensor_tensor(out=acc_t[:], in0=acc_t[:],
                                        in1=ptmp[:], op=ADD)

                if not last_layer:
                    # next-layer scaled weights + rank-1 row
                    nc.vector.tensor_scalar(
                        out=w1f[:], in0=w1_t[:, (L + 1) * DIM:(L + 2) * DIM],
                        scalar1=s_t[:], scalar2=None, op0=MULT)
                    c_ps = psP.tile([DIM, 1], f32, name="c_ps", tag="pout")
                    nc.tensor.matmul(
                        c_ps[:], lhsT=w1_t[:, (L + 1) * DIM:(L + 2) * DIM],
                        rhs=t_t[:], start=True, stop=True)
                    c_col = cp.tile([DIM, 1], f32, name=f"ccol{L}")
                    nc.vector.tensor_copy(c_col[:], c_ps[:])
                    nc.sync.dma_start(c_dram.ap()[:], c_col[:, 0])
                    nc.sync.dma_start(c_row[:], c_dram.ap()[:].unsqueeze(0))

            nc.sync.dma_start(acc_out.ap()[:, :], acc_t[:])
    nc.compile()
    return nc


def kernel(x, edge_index, batch, percent, ini_w1, ini_b1, ini_w2, ini_b2,
           gin_w1, gin_b1, gin_w2, gin_b2, bn_gamma, bn_beta, lc_w, lc_b):
    x = np.asarray(x, np.float32)
    src = np.asarray(edge_index[0], np.int64)
    dst = np.asarray(edge_index[1], np.int64)
    batch = np.asarray(batch, np.int64)

    percore, T, PW, I_full, I_last, mask, iota = _prep(src, dst, batch)

    key = (T, PW)
    if key not in _CACHE:
        _CACHE[key] = _build(T, PW)
    nc = _CACHE[key]

    bf = ml_dtypes.bfloat16
    com = dict(
        If=I_full.astype(bf), Il=I_last.astype(bf), mask=mask,
        iota=iota.astype(bf),
        iw1=np.asarray(ini_w1, np.float32),
        ib1=np.asarray(ini_b1, np.float32).reshape(DIM, 1),
        iw2=np.asarray(ini_w2, np.float32),
        ib2=np.asarray(ini_b2, np.float32).reshape(1, DIM),
        w1=np.concatenate([np.asarray(gin_w1[i], np.float32)
                           for i in range(N_LAYERS)], axis=1),
        w2=np.concatenate([np.asarray(gin_w2[i], np.float32)
                           for i in range(N_LAYERS)], axis=1),
        b1T=np.asarray(gin_b1, np.float32).T.copy(),
        b2r=np.asarray(gin_b2, np.float32).reshape(1, N_LAYERS * DIM),
        gamT=np.asarray(bn_gamma, np.float32).T.copy(),
        betT=np.asarray(bn_beta, np.float32).T.copy(),
        lcwb=np.repeat(np.asarray(lc_w, np.float32).reshape(1, N_LAYERS),
                       DIM, 0),
    )
    in_maps = []
    for c in range(NC):
        pc = percore[c]
        m = dict(com)
        m["xT"] = x[c * NLOC:(c + 1) * NLOC].T.copy()
        m["idx"] = pc["idx"]
        m["doff"] = pc["doff"]
        m["degp"] = pc["degp"]
        m["Sp"] = pc["Sp"]
        in_maps.append(m)

    trace = os.environ.get("KERNEL_TRACE", "0") == "1"
    res = bass_utils.run_bass_kernel_spmd(
        nc, in_maps, core_ids=list(range(NC)), trace=trace)
    global _LAST_RES
    _LAST_RES = res

    # ---- host unshard/combine
    lcw = np.asarray(lc_w, np.float32)
    lcb = np.float32(np.asarray(lc_b, np.float32))
    st = res.results[0]["st"]            # [2L, 128]
    t_all = st[1::2]                     # [L, 128]
    cnt = np.bincount(batch, minlength=N_GRAPHS).astype(np.float32)

    out = np.zeros((N_GRAPHS, DIM), np.float32)
    for c in range(NC):
        acc = res.results[c]["acc"]      # [128, NT*PW]
        glo = percore[c]["glo"]
        for t in range(NT):
            g0 = int(glo[t])
            w = min(PW - 1, N_GRAPHS - g0)
            out[g0:g0 + w, :] += acc[:, t * PW:t * PW + w].T

    tsum = (lcw[:, None] * t_all).sum(0) + lcb   # [128]
    out += cnt[:, None] * tsum[None, :]
    return out


# revision 44
# speedup vs baseline: 1.0160x; 1.0160x over previous
"""GIN encoder (5-layer GNN + BN + global pooling) on 8 TRN2 NeuronCores.

kernel(**inputs) takes FULL inputs, returns FULL [8192, 128] output.

v2 design (SPMD, one bass program, per-core data):
- Nodes in 8 contiguous 25000-row shards; activations exchanged per layer
  via CHUNKED AllGather (4 slices, overlapped with tile compute) into a
  chunk-interleaved bf16 table [200000,128]; layer 4 skips the exchange.
- Aggregation per 4-tile group into a [128,512] PSUM: gathered src rows
  via batched multi-chunk indirect DMA (12 chunks per instruction, 3D out
  AP, round-robin over 4 SWDGE queues); 0/1 selection matrices generated
  ON-CHIP per group with one DVE is_equal (broadcast AP) from dst offsets;
  self term via identity matmul from the previous-layer zbuf (batched
  stage loads).
- Delayed BN: w1 pre-scaled by s; the deg'*(W1^T t) rank-1 term lands in
  the MLP1 PSUM as one K=1 matmul per group; BN stats from Gram diagonal
  + pooled ones column, AllReduced.
- Pooling accumulated on-chip across layers (acc += lcw*s (.) pall_L),
  written once; host adds the cnt*(sum lcw*t + lcb) term and overlap-adds
  per-tile windows.
"""
import os
import numpy as np
import ml_dtypes

from concourse import bass, bacc, tile, mybir
from concourse import bass_utils

N_NODES = 200000
N_EDGES = 400000
N_FEAT = 78
DIM = 128
N_LAYERS = 5
N_GRAPHS = 8192
BN_EPS = 1e-5
NC = 8
NLOC = N_NODES // NC
NT = (NLOC + 127) // 128          # 196 tiles of 128 nodes
NG4 = (NT + 3) // 4               # 49 groups of 4 tiles
AGCH = 4                          # AllGather chunks per layer
CHROW = NLOC // AGCH              # 6250 rows per AG chunk per core
OOB = 1 << 30
PAD_DOFF = 200.0

f32 = mybir.dt.float32
bf16 = mybir.dt.bfloat16
i32 = mybir.dt.int32
Relu = mybir.ActivationFunctionType.Relu
Copy = mybir.ActivationFunctionType.Copy
Sqrt = mybir.ActivationFunctionType.Sqrt
ADD = mybir.AluOpType.add
MULT = mybir.AluOpType.mult
SUB = mybir.AluOpType.subtract
ISEQ = mybir.AluOpType.is_equal

_CACHE = {}
_LAST_RES = None


def _remap(g):
    """global node id -> row in the chunk-interleaved AG table."""
    c2 = g // NLOC
    r = g % NLOC
    k = r // CHROW
    return k * (NC * CHROW) + c2 * CHROW + (r % CHROW)


def _prep(src, dst, batch):
    order = np.argsort(dst, kind="stable")
    src_s = src[order].astype(np.int64)
    dst_s = dst[order].astype(np.int64)

    cores_e = []
    T = 0
    for c in range(NC):
        lo = c * NLOC
        m = (dst_s >= lo) & (dst_s < lo + NLOC)
        es, ed = src_s[m], dst_s[m] - lo
        cnt = np.bincount(ed // 128, minlength=NT)
        T = max(T, int(np.ceil(cnt.max() / 128)))
        cores_e.append((es, ed, cnt))

    PW = 0
    glo_all = []
    for c in range(NC):
        b = batch[c * NLOC:(c + 1) * NLOC]
        glo = np.zeros(NT, np.int64)
        for t in range(NT):
            seg = b[t * 128: min((t + 1) * 128, NLOC)]
            glo[t] = seg[0]
            PW = max(PW, int(seg[-1] - seg[0] + 1))
        glo_all.append(glo)
    PW += 1  # ones column

    percore = []
    for c in range(NC):
        es, ed, cnt = cores_e[c]
        NCH = NT * T
        idx = np.full((NCH, 128), OOB, np.int64)
        doff = np.full((NCH, 128), PAD_DOFF, np.float32)
        off = np.concatenate([[0], np.cumsum(cnt)])
        for t in range(NT):
            e0, e1 = int(off[t]), int(off[t + 1])
            r = np.arange(e1 - e0)
            idx[t * T + r // 128, r % 128] = _remap(es[e0:e1])
            doff[t * T + r // 128, r % 128] = ed[e0:e1] - t * 128
        idx = np.minimum(idx, OOB).astype(np.int32)

        degp = np.zeros(NG4 * 512, np.float32)
        dcnt = np.bincount(ed, minlength=NLOC).astype(np.float32)
        degp[:NLOC] = dcnt + 1.0
        degp = degp.reshape(NG4, 512)

        b = batch[c * NLOC:(c + 1) * NLOC]
        Sp = np.zeros((128, NT * PW), np.float32)
        glo = glo_all[c]
        for t in range(NT):
            n0, n1 = t * 128, min(t * 128 + 128, NLOC)
            p = np.arange(n1 - n0)
            Sp[p, t * PW + (b[n0:n1] - glo[t])] = 1.0
            Sp[p, t * PW + PW - 1] = 1.0

        percore.append(dict(
            idx=idx.T.copy(),                      # [128, NCH]
            doff=doff.T.astype(ml_dtypes.bfloat16).copy(),   # [128, NCH]
            degp=degp.astype(np.float32),          # [NG4, 512]
            Sp=Sp.astype(ml_dtypes.bfloat16),
            glo=glo))

    I_full = np.eye(128, dtype=np.float32)
    I_last = np.zeros((128, 128), np.float32)
    base = (NT - 1) * 128 - (NLOC - 128)   # slot offset of first last-tile row
    for j in range(NLOC - (NT - 1) * 128):
        I_last[base + j, j] = 1.0

    mask = np.zeros((128, NT), np.float32)
    for t in range(NT):
        mask[:min(128, NLOC - t * 128), t] = 1.0

    iota = np.tile(np.arange(128, dtype=np.float32), 4 * T)
    iota = np.repeat(iota.reshape(1, -1), 128, 0)

    return percore, T, PW, I_full, I_last, mask, iota


def _gather_q(nc, out_ap, in_ap, offset_ap, bounds, qnum):
    """Batched indirect row-gather with SWDGE queue selection.

    out_ap must be 3D [128, K, elem] so each 128-elem run gets its own
    index from offset_ap [128, K]."""
    g = nc.gpsimd
    out_l = g.lower_ap_dma(out_ap, for_indirect_dma=True)
    in_l = g.lower_ap_dma(in_ap, for_indirect_dma=True)
    assert len(in_l) == 1 and len(out_l) == 1
    off_l = g.lower_ap_dma(offset_ap)
    assert len(off_l) == 1
    in_l.append(off_l[0])
    ap_shape = in_ap.shape
    coef = 1
    for i in range(1, len(ap_shape)):
        coef *= ap_shape[i]
    in_l[0].dynamic_ap_info = mybir.DynamicAccessPatternInfo(
        c=0,
        actual_ap=out_ap.ap,
        indirect_dim_max_index=ap_shape[0],
        offset_expr=[
            mybir.DynamicAccessPatternOffsetExpr(
                coef=coef,
                aff_expr=mybir.DynamicAccessPatternOffsetExprAffExpr(
                    kind="IndirectArgId", arg_id=1,
                ),
            )
        ],
    )
    bc = [g.lower_val_access(g.to_reg(bounds))] if bounds is not None else []
    return g.add_instruction(
        mybir.InstDMACopy(
            name=g.bass.get_next_instruction_name(),
            queue=f"qPoolDynamic{qnum or ''}",
            mode="Copy",
            ins=in_l + bc,
            outs=out_l,
            oob_is_err=False,
            cce_op=mybir.AluOpType.bypass,
        )
    )


def _build(T, PW):
    nc = bacc.Bacc("TRN2", target_bir_lowering=False, debug=False,
                   num_devices=NC, num_swdge_queues=4)
    NCH = NT * T
    L5 = N_LAYERS
    GW = 4 * T          # gather chunks per 4-tile group

    xT = nc.dram_tensor("xT", [N_FEAT, NLOC], f32, kind="ExternalInput")
    idx_in = nc.dram_tensor("idx", [128, NCH], i32, kind="ExternalInput")
    doff_in = nc.dram_tensor("doff", [128, NCH], bf16, kind="ExternalInput")
    iota_in = nc.dram_tensor("iota", [128, GW * 128], bf16,
                             kind="ExternalInput")
    degp_in = nc.dram_tensor("degp", [NG4, 512], f32, kind="ExternalInput")
    Sp_in = nc.dram_tensor("Sp", [128, NT * PW], bf16, kind="ExternalInput")
    If_in = nc.dram_tensor("If", [128, 128], bf16, kind="ExternalInput")
    Il_in = nc.dram_tensor("Il", [128, 128], bf16, kind="ExternalInput")
    mask_in = nc.dram_tensor("mask", [128, NT], f32, kind="ExternalInput")
    iw1_in = nc.dram_tensor("iw1", [N_FEAT, DIM], f32, kind="ExternalInput")
    ib1_in = nc.dram_tensor("ib1", [DIM, 1], f32, kind="ExternalInput")
    iw2_in = nc.dram_tensor("iw2", [DIM, DIM], f32, kind="ExternalInput")
    ib2_in = nc.dram_tensor("ib2", [1, DIM], f32, kind="ExternalInput")
    w1_in = nc.dram_tensor("w1", [DIM, L5 * DIM], f32, kind="ExternalInput")
    w2_in = nc.dram_tensor("w2", [DIM, L5 * DIM], f32, kind="ExternalInput")
    b1T_in = nc.dram_tensor("b1T", [DIM, L5], f32, kind="ExternalInput")
    b2r_in = nc.dram_tensor("b2r", [1, L5 * DIM], f32, kind="ExternalInput")
    gamT_in = nc.dram_tensor("gamT", [DIM, L5], f32, kind="ExternalInput")
    betT_in = nc.dram_tensor("betT", [DIM, L5], f32, kind="ExternalInput")
    lcw_in = nc.dram_tensor("lcwb", [DIM, L5], f32, kind="ExternalInput")

    acc_out = nc.dram_tensor("acc", [DIM, NT * PW], f32,
                             kind="ExternalOutput")
    st_out = nc.dram_tensor("st", [L5 * 2, DIM], f32, kind="ExternalOutput")

    tables = [nc.dram_tensor(f"table{i}", [N_NODES, DIM], bf16,
                             kind="Internal", addr_space="Shared")
              for i in range(2)]
    zbuf = [nc.dram_tensor(f"zbuf{i}", [NLOC, DIM], bf16, kind="Internal")
            for i in range(2)]
    ar_in = nc.dram_tensor("ar_in", [DIM, 2], f32, kind="Internal")
    ar_out = nc.dram_tensor("ar_out", [DIM, 2], f32, kind="Internal",
                            addr_space="Shared")
    c_dram = nc.dram_tensor("c_dram", [DIM], f32, kind="Internal")
    RG = [list(range(NC))]

    with tile.TileContext(nc) as tc:
        with tc.tile_pool(name="const", bufs=1) as cp, \
             tc.tile_pool(name="gpool", bufs=8) as gpool, \
             tc.tile_pool(name="spool", bufs=3) as spool, \
             tc.tile_pool(name="gsp", bufs=2) as gsp, \
             tc.tile_pool(name="z1p", bufs=2) as z1p, \
             tc.tile_pool(name="y1p", bufs=2) as y1p, \
             tc.tile_pool(name="zsp", bufs=2) as zsp, \
             tc.tile_pool(name="xbp", bufs=2) as xbp, \
             tc.tile_pool(name="dgp", bufs=2) as dgp, \
             tc.tile_pool(name="pallp", bufs=2) as pallp, \
             tc.tile_pool(name="psA", bufs=2, space="PSUM") as psA, \
             tc.tile_pool(name="psB", bufs=2, space="PSUM") as psB, \
             tc.tile_pool(name="psC", bufs=2, space="PSUM") as psC, \
             tc.tile_pool(name="psG", bufs=1, space="PSUM") as psG, \
             tc.tile_pool(name="psP", bufs=1, space="PSUM") as psP:

            def ld(shape, dt_, src_ap, name):
                t_ = cp.tile(shape, dt_, name=name)
                nc.sync.dma_start(t_[:], src_ap)
                return t_

            idx_t = ld([128, NCH], i32, idx_in[:], "idx_t")
            doff_t = ld([128, NCH], bf16, doff_in[:], "doff_t")
            iota_t = ld([128, GW * 128], bf16, iota_in[:], "iota_t")
            Sp_t = ld([128, NT * PW], bf16, Sp_in[:], "Sp_t")
            If_t = ld([128, 128], bf16, If_in[:], "If_t")
            Il_t = ld([128, 128], bf16, Il_in[:], "Il_t")
            mask_t = ld([128, NT], f32, mask_in[:], "mask_t")
            iw1_t = ld([N_FEAT, DIM], f32, iw1_in[:], "iw1_t")
            ib1_t = ld([DIM, 1], f32, ib1_in[:], "ib1_t")
            iw2_t = ld([DIM, DIM], f32, iw2_in[:], "iw2_t")
            ib2_t = ld([1, DIM], f32, ib2_in[:], "ib2_t")
            iw2b = cp.tile([DIM, DIM], bf16, name="iw2b")
            nc.vector.tensor_copy(iw2b[:], iw2_t[:])
            ib2b = cp.tile([1, DIM], bf16, name="ib2b")
            nc.vector.tensor_copy(ib2b[:], ib2_t[:])
            w1_t = ld([DIM, L5 * DIM], f32, w1_in[:], "w1_t")
            w2_t = ld([DIM, L5 * DIM], f32, w2_in[:], "w2_t")
            b1T_t = ld([DIM, L5], f32, b1T_in[:], "b1T_t")
            b2r_t = ld([1, L5 * DIM], f32, b2r_in[:], "b2r_t")
            gamT_t = ld([DIM, L5], f32, gamT_in[:], "gamT_t")
            betT_t = ld([DIM, L5], f32, betT_in[:], "betT_t")
            lcw_t = ld([DIM, L5], f32, lcw_in[:], "lcw_t")

            ident_f = cp.tile([128, 128], f32, name="ident_f")
            nc.vector.tensor_copy(ident_f[:], If_t[:])
            ones_r = cp.tile([1, 128], bf16, name="ones_r")
            nc.vector.memset(ones_r[:], 1.0)
            w2b_all = cp.tile([DIM, L5 * DIM], bf16, name="w2b_all")
            nc.vector.tensor_copy(w2b_all[:], w2_t[:])
            b2rb = cp.tile([1, L5 * DIM], bf16, name="b2rb")
            nc.vector.tensor_copy(b2rb[:], b2r_t[:])

            w1f = cp.tile([DIM, DIM], f32, name="w1f")
            nc.vector.tensor_copy(w1f[:], w1_t[:, 0:DIM])
            c_row = cp.tile([1, DIM], f32, name="c_row")
            acc_t = cp.tile([DIM, NT * PW], f32, name="acc_t")
            nc.vector.memset(acc_t[:], 0.0)

            for _ in range(8):
                g0 = gpool.tile([128, 128], bf16, name="g", tag="g")
                nc.vector.memset(g0[:], 0.0)

            def flush_z(zdst, ck0, ntile, zstage):
                """DMA zstage [128, ntile*128] -> zdst rows."""
                r0 = ck0 * 128
                full = min(ntile, (NLOC - r0) // 128)
                if full > 0:
                    nc.sync.dma_start(
                        zdst.ap()[r0:r0 + full * 128, :].rearrange(
                            "(j p) f -> p j f", p=128),
                        zstage[:, :full * 128].rearrange(
                            "p (j f) -> p j f", j=full))
                rem = NLOC - (r0 + full * 128)
                if 0 < rem < 128 and full < ntile:
                    nc.sync.dma_start(
                        zdst.ap()[r0 + full * 128:NLOC, :],
                        zstage[:rem, full * 128:(full + 1) * 128])

            # ---------------- ini embed -> zbuf[0], table ----------------
            ag_done = 0

            def maybe_ag(zdst, tdst, rows_done, last_layer):
                nonlocal ag_done
                if last_layer:
                    return
                while ag_done < AGCH and rows_done >= (ag_done + 1) * CHROW:
                    k = ag_done
                    nc.gpsimd.collective_compute(
                        "AllGather", mybir.AluOpType.bypass,
                        replica_groups=RG,
                        ins=[zdst.ap()[k * CHROW:(k + 1) * CHROW, :]],
                        outs=[tdst.ap()[k * NC * CHROW:
                                        (k + 1) * NC * CHROW, :]])
                    ag_done += 1

            for gidx in range(NG4):
                n0 = gidx * 512
                w = min(512, NLOC - n0)
                nt4 = (w + 127) // 128
                xb = xbp.tile([N_FEAT, 512], f32, name="xb", tag="xb")
                nc.sync.dma_start(xb[:, :w], xT[:, n0:n0 + w])
                yp = psB.tile([DIM, 512], f32, name="yp", tag="yp")
                nc.tensor.matmul(yp[:, :w], lhsT=iw1_t[:], rhs=xb[:, :w],
                                 start=True, stop=True)
                y1b = y1p.tile([DIM, 512], bf16, name="y1b", tag="y1")
                nc.scalar.activation(y1b[:, :w], yp[:, :w], Relu,
                                     bias=ib1_t[:], scale=1.0)
                zstage = zsp.tile([128, 512], bf16, name="zstage", tag="zs")
                for k in range(nt4):
                    cw = min(128, w - k * 128)
                    zp = psC.tile([128, DIM], f32, name="zp", tag="zp")
                    nc.tensor.matmul(zp[:cw, :],
                                     lhsT=y1b[:, k * 128:k * 128 + cw],
                                     rhs=iw2b[:], start=True, stop=False)
                    nc.tensor.matmul(zp[:cw, :], lhsT=ones_r[:, :cw],
                                     rhs=ib2b[:], start=False, stop=True)
                    nc.scalar.activation(
                        zstage[:, k * 128:(k + 1) * 128], zp[:], Copy,
                        scale=mask_t[:, gidx * 4 + k:gidx * 4 + k + 1])
                flush_z(zbuf[0], gidx * 4, nt4, zstage)
                maybe_ag(zbuf[0], tables[0], min(n0 + 512, NLOC), False)

            # ---------------- layers ----------------
            for L in range(L5):
                zprev = zbuf[L % 2]
                zcur = zbuf[(L + 1) % 2]
                last_layer = (L == L5 - 1)
                ag_done = 0
                gram = psG.tile([128, 128], f32, name="gram", tag="gram")
                pall = pallp.tile([DIM, NT * PW], f32, name="pall",
                                  tag="pall")

                for gidx in range(NG4):
                    ck0 = gidx * 4
                    nt4 = min(4, NT - ck0)
                    # gathers: one indirect DMA per 128-row chunk (deep
                    # pipelining via the rotating per-chunk pool)
                    gtiles = []
                    for k in range(GW):
                        ch = ck0 * T + k
                        gt = gpool.tile([128, 128], bf16, name="g", tag="g")
                        _gather_q(
                            nc, gt[:],
                            tables[L % 2][:],
                            idx_t[:, ch:ch + 1],
                            N_NODES - 1, 0)
                        gtiles.append(gt)
                    # S for this group (on-chip is_equal)
                    St = spool.tile([128, GW * 128], bf16, name="St",
                                    tag="S")
                    nc.vector.tensor_tensor(
                        out=St[:].rearrange("p (k f) -> p k f", k=GW),
                        in0=iota_t[:].rearrange("p (k f) -> p k f", k=GW),
                        in1=doff_t[:, ck0 * T: ck0 * T + GW].unsqueeze(
                            2).broadcast_to([128, GW, 128]),
                        op=ISEQ)
                    # self rows
                    gs = gsp.tile([128, 512], bf16, name="gs", tag="gs")
                    if ck0 * 128 + 512 <= NLOC:
                        nc.sync.dma_start(
                            gs[:].rearrange("p (j f) -> p j f", j=4),
                            zprev.ap()[ck0 * 128:ck0 * 128 + 512, :]
                            .rearrange("(j p) f -> p j f", p=128))
                    else:
                        for j in range(nt4):
                            t_ = ck0 + j
                            r0 = min(t_ * 128, NLOC - 128)
                            nc.sync.dma_start(
                                gs[:, j * 128:(j + 1) * 128],
                                zprev.ap()[r0:r0 + 128, :])
                    u = psA.tile([128, 512], f32, name="u", tag="u")
                    for j in range(nt4):
                        t_ = ck0 + j
                        for k in range(T):
                            nc.tensor.matmul(
                                u[:, j * 128:(j + 1) * 128],
                                lhsT=gtiles[j * T + k][:],
                                rhs=St[:, (j * T + k) * 128:
                                       (j * T + k + 1) * 128],
                                start=(k == 0), stop=False)
                        It = Il_t if t_ == NT - 1 else If_t
                        nc.tensor.matmul(
                            u[:, j * 128:(j + 1) * 128],
                            lhsT=gs[:, j * 128:(j + 1) * 128],
                            rhs=It[:], start=False, stop=True)
                    bw = nt4 * 128
                    z1b = z1p.tile([DIM, 512], f32, name="z1b", tag="z1")
                    nc.scalar.activation(z1b[:, :bw], u[:, :bw], Copy,
                                         scale=1.0)
                    yp = psB.tile([DIM, 512], f32, name="yp", tag="yp")
                    nc.tensor.matmul(yp[:, :bw], lhsT=w1f[:],
                                     rhs=z1b[:, :bw], start=True,
                                     stop=(L == 0))
                    if L > 0:
                        dg = dgp.tile([1, 512], f32, name="dg", tag="dg")
                        nc.sync.dma_start(dg[:], degp_in[gidx:gidx + 1, :])
                        nc.tensor.matmul(yp[:, :bw], lhsT=c_row[:],
                                         rhs=dg[:, :bw], start=False,
                                         stop=True)
                    y1b = y1p.tile([DIM, 512], bf16, name="y1b", tag="y1")
                    nc.scalar.activation(y1b[:, :bw], yp[:, :bw], Relu,
                                         bias=b1T_t[:, L:L + 1], scale=1.0)
                    zstage = zsp.tile([128, 512], bf16, name="zstage",
                                      tag="zs")
                    for j in range(nt4):
                        ck = ck0 + j
                        zp = psC.tile([128, DIM], f32, name="zp", tag="zp")
                        nc.tensor.matmul(
                            zp[:], lhsT=y1b[:, j * 128:(j + 1) * 128],
                            rhs=w2b_all[:, L * DIM:(L + 1) * DIM],
                            start=True, stop=False)
                        nc.tensor.matmul(
                            zp[:], lhsT=ones_r[:],
                            rhs=b2rb[:, L * DIM:(L + 1) * DIM],
                            start=False, stop=True)
                        nc.scalar.activation(
                            zstage[:, j * 128:(j + 1) * 128], zp[:], Relu,
                            scale=mask_t[:, ck:ck + 1])
                        zts = zstage[:, j * 128:(j + 1) * 128]
                        pout = psP.tile([128, PW], f32, name="pout",
                                        tag="pout")
                        nc.tensor.matmul(
                            pout[:], lhsT=zts,
                            rhs=Sp_t[:, ck * PW:(ck + 1) * PW],
                            start=True, stop=True)
                        nc.vector.tensor_copy(
                            pall[:, ck * PW:(ck + 1) * PW], pout[:])
                        nc.tensor.matmul(gram[:], lhsT=zts, rhs=zts,
                                         start=(ck == 0),
                                         stop=(ck == NT - 1))
                    if not last_layer:
                        flush_z(zcur, ck0, nt4, zstage)
                        maybe_ag(zcur, tables[(L + 1) % 2],
                                 min((gidx + 1) * 512, NLOC), last_layer)

                # ---- stats
                zsum = cp.tile([DIM, 1], f32, name=f"zsum{L}")
                nc.vector.tensor_reduce(
                    out=zsum[:],
                    in_=pall[:].rearrange("p (t w) -> p t w",
                                          w=PW)[:, :, PW - 1],
                    op=ADD, axis=mybir.AxisListType.X)
                gmul = cp.tile([128, 128], f32, name=f"gmul{L}")
                nc.vector.tensor_tensor(out=gmul[:], in0=gram[:],
                                        in1=ident_f[:], op=MULT)
                zsq = cp.tile([DIM, 1], f32, name=f"zsq{L}")
                nc.vector.tensor_reduce(out=zsq[:], in_=gmul[:], op=ADD,
                                        axis=mybir.AxisListType.X)
                stats_sb = cp.tile([DIM, 2], f32, name=f"stats{L}")
                nc.vector.tensor_copy(stats_sb[:, 0:1], zsum[:])
                nc.vector.tensor_copy(stats_sb[:, 1:2], zsq[:])
                nc.sync.dma_start(ar_in.ap()[:, :], stats_sb[:])
                nc.gpsimd.collective_compute(
                    "AllReduce", ADD, replica_groups=RG,
                    ins=[ar_in.ap()], outs=[ar_out.ap()])
                arst = cp.tile([DIM, 2], f32, name=f"arst{L}")
                nc.sync.dma_start(arst[:], ar_out.ap()[:, :])
                mean = cp.tile([DIM, 1], f32, name=f"mean{L}")
                nc.vector.tensor_scalar(out=mean[:], in0=arst[:, 0:1],
                                        scalar1=1.0 / N_NODES, scalar2=None,
                                        op0=MULT)
                ex2 = cp.tile([DIM, 1], f32, name=f"ex2{L}")
                nc.vector.tensor_scalar(out=ex2[:], in0=arst[:, 1:2],
                                        scalar1=1.0 / N_NODES, scalar2=None,
                                        op0=MULT)
                m2 = cp.tile([DIM, 1], f32, name=f"m2{L}")
                nc.vector.tensor_tensor(out=m2[:], in0=mean[:], in1=mean[:],
                                        op=MULT)
                var = cp.tile([DIM, 1], f32, name=f"var{L}")
                nc.vector.tensor_tensor(out=var[:], in0=ex2[:], in1=m2[:],
                                        op=SUB)
                vare = cp.tile([DIM, 1], f32, name=f"vare{L}")
                nc.vector.tensor_scalar(out=vare[:], in0=var[:],
                                        scalar1=BN_EPS, scalar2=None,
                                        op0=ADD)
                sstd = cp.tile([DIM, 1], f32, name=f"sstd{L}")
                nc.scalar.activation(sstd[:], vare[:], Sqrt, bias=0.0,
                                     scale=1.0)
                rinv = cp.tile([DIM, 1], f32, name=f"rinv{L}")
                nc.vector.reciprocal(rinv[:], sstd[:])
                s_t = cp.tile([DIM, 1], f32, name=f"s{L}")
                nc.vector.tensor_tensor(out=s_t[:], in0=rinv[:],
                                        in1=gamT_t[:, L:L + 1], op=MULT)
                ms = cp.tile([DIM, 1], f32, name=f"ms{L}")
                nc.vector.tensor_tensor(out=ms[:], in0=mean[:], in1=s_t[:],
                                        op=MULT)
                t_t = cp.tile([DIM, 1], f32, name=f"t{L}")
                nc.vector.tensor_tensor(out=t_t[:], in0=betT_t[:, L:L + 1],
                                        in1=ms[:], op=SUB)
                nc.sync.dma_start(st_out.ap()[2 * L, :], s_t[:, 0])
                nc.sync.dma_start(st_out.ap()[2 * L + 1, :], t_t[:, 0])

                # fold pall into acc with lcw[L]*s
                fcol = cp.tile([DIM, 1], f32, name=f"fcol{L}")
                nc.vector.tensor_tensor(out=fcol[:], in0=s_t[:],
                                        in1=lcw_t[:, L:L + 1], op=MULT)
                ptmp = pallp.tile([DIM, NT * PW], f32, name="ptmp",
                                  tag="pall")
                nc.vector.tensor_scalar(out=ptmp[:], in0=pall[:],
                                        scalar1=fcol[:], scalar2=None,
                                        op0=MULT)
                nc.vector.tensor_tensor(out=acc_t[:], in0=acc_t[:],
                                        in1=ptmp[:], op=ADD)

                if not last_layer:
                    # next-layer scaled weights + rank-1 row
                    nc.vector.tensor_scalar(
                        out=w1f[:], in0=w1_t[:, (L + 1) * DIM:(L + 2) * DIM],
                        scalar1=s_t[:], scalar2=None, op0=MULT)
                    c_ps = psP.tile([DIM, 1], f32, name="c_ps", tag="pout")
                    nc.tensor.matmul(
                        c_ps[:], lhsT=w1_t[:, (L + 1) * DIM:(L + 2) * DIM],
                        rhs=t_t[:], start=True, stop=True)
                    c_col = cp.tile([DIM, 1], f32, name=f"ccol{L}")
                    nc.vector.tensor_copy(c_col[:], c_ps[:])
                    nc.sync.dma_start(c_dram.ap()[:], c_col[:, 0])
                    nc.sync.dma_start(c_row[:], c_dram.ap()[:].unsqueeze(0))

            nc.sync.dma_start(acc_out.ap()[:, :], acc_t[:])
    nc.compile()
    return nc


def kernel(x, edge_index, batch, percent, ini_w1, ini_b1, ini_w2, ini_b2,
           gin_w1, gin_b1, gin_w2, gin_b2, bn_gamma, bn_beta, lc_w, lc_b):
    x = np.asarray(x, np.float32)
    src = np.asarray(edge_index[0], np.int64)
    dst = np.asarray(edge_index[1], np.int64)
    batch = np.asarray(batch, np.int64)

    percore, T, PW, I_full, I_last, mask, iota = _prep(src, dst, batch)

    key = (T, PW)
    if key not in _CACHE:
        _CACHE[key] = _build(T, PW)
    nc = _CACHE[key]

    bf = ml_dtypes.bfloat16
    com = dict(
        If=I_full.astype(bf), Il=I_last.astype(bf), mask=mask,
        iota=iota.astype(bf),
        iw1=np.asarray(ini_w1, np.float32),
        ib1=np.asarray(ini_b1, np.float32).reshape(DIM, 1),
        iw2=np.asarray(ini_w2, np.float32),
        ib2=np.asarray(ini_b2, np.float32).reshape(1, DIM),
        w1=np.concatenate([np.asarray(gin_w1[i], np.float32)
                           for i in range(N_LAYERS)], axis=1),
        w2=np.concatenate([np.asarray(gin_w2[i], np.float32)
                           for i in range(N_LAYERS)], axis=1),
        b1T=np.asarray(gin_b1, np.float32).T.copy(),
        b2r=np.asarray(gin_b2, np.float32).reshape(1, N_LAYERS * DIM),
        gamT=np.asarray(bn_gamma, np.float32).T.copy(),
        betT=np.asarray(bn_beta, np.float32).T.copy(),
        lcwb=np.repeat(np.asarray(lc_w, np.float32).reshape(1, N_LAYERS),
                       DIM, 0),
    )
    in_maps = []
    for c in range(NC):
        pc = percore[c]
        m = dict(com)
        m["xT"] = x[c * NLOC:(c + 1) * NLOC].T.copy()
        m["idx"] = pc["idx"]
        m["doff"] = pc["doff"]
        m["degp"] = pc["degp"]
        m["Sp"] = pc["Sp"]
        in_maps.append(m)

    trace = os.environ.get("KERNEL_TRACE", "0") == "1"
    res = bass_utils.run_bass_kernel_spmd(
        nc, in_maps, core_ids=list(range(NC)), trace=trace)
    global _LAST_RES
    _LAST_RES = res

    # ---- host unshard/combine
    lcw = np.asarray(lc_w, np.float32)
    lcb = np.float32(np.asarray(lc_b, np.float32))
    st = res.results[0]["st"]            # [2L, 128]
    t_all = st[1::2]                     # [L, 128]
    cnt = np.bincount(batch, minlength=N_GRAPHS).astype(np.float32)

    out = np.zeros((N_GRAPHS, DIM), np.float32)
    for c in range(NC):
        acc = res.results[c]["acc"]      # [128, NT*PW]
        glo = percore[c]["glo"]
        for t in range(NT):
            g0 = int(glo[t])
            w = min(PW - 1, N_GRAPHS - g0)
            out[g0:g0 + w, :] += acc[:, t * PW:t * PW + w].T

    tsum = (lcw[:, None] * t_all).sum(0) + lcb   # [128]
    out += cnt[:, None] * tsum[None, :]
    return out
